# revision 1
# baseline (speedup 1.0000x reference)
"""CapsuleLayer dynamic-routing kernel for Trainium2, SPMD across 8 NeuronCores.

Reference computation (all fp32):
  u_hat[b,c,u,j] = sum_i W[c,u,j,i] * x[b,i,c]
  b_ij = 0;  3 routing iterations:
    c_ij = softmax_u(b_ij)                         # [C,U], batch-independent
    s[b,u,j] = sum_c c_ij[c,u] * u_hat[b,c,u,j]
    v = squash_j(s)
    u_vj[c,u] = mean_b <u_hat[b,c,u,:], v[b,u,:]>  # iters 0,1 only
    b_ij += u_vj
  return v from the last iteration, shaped [B, U, J, 1].

Distribution: the C=1152 input-capsule axis is sharded 144 per core
(capsule-parallel rather than the hinted batch-parallel: it shards the
weight-side work and load 8x, makes the b_ij update fully local, and needs
only one collective per routing iteration).  u_hat is never materialized:

  s_partial[b,uj] = x_ci^T @ (W_s * bcast(c_ij))      (K = (c,i) = 1152/core)
     -> AllReduce over the 8 cores (iters 0,1), ReduceScatter (iter 2)
  agreement: M2[(ci),(uj)] = x_bt^T @ v  (K = b), then
     u_vj[c,u] = sum_{i,j} W_s * M2   (j-reduce on DVE; the i-reduce runs
     across partitions via a 0/1 selector matmul on the PE)

Each core returns its 32-row batch shard of v (from the ReduceScatter);
the host concatenates the shards.
"""
import numpy as np
import concourse.bacc as bacc
import concourse.mybir as mybir
import concourse.tile as tile
from concourse.bass_utils import run_bass_kernel_spmd
from concourse.alu_op_type import AluOpType

F32 = mybir.dt.float32
F32R = mybir.dt.float32r
AF = mybir.ActivationFunctionType
AX = mybir.AxisListType

B = 256          # batch
C = 1152         # input capsules
I = 8            # input capsule dim
U = 10           # output capsules
J = 16           # output capsule dim
UJ = U * J       # 160
NCORES = 8
CSH = C // NCORES     # 144 capsules per core
CI = CSH * I          # 1152 (c,i) contraction rows per core
KT = CI // 128        # 9 contraction tiles
BT = B // 128         # 2 batch tiles
BSH = B // NCORES     # 32 batch rows per core after the final ReduceScatter
NCH = CSH // 16       # 9 c-chunks of 16 (b_ij lives as [16, NCH*U])

# Every ACT function this kernel uses (Exp, Ln, Square) lives in the
# 'natural_log_exp_and_others' table (Sqrt, used once on the output path, in
# 'sqrt_and_others').  The stock chooser assigns each activation the first
# table containing its function, which alternates tables between softmax
# (Exp) and squash (Ln) and pays a ~1.3us table load per switch.  Restrict
# the candidate list (preserving positions == act_func_set_ids) so a single
# hoisted load per table suffices.
_orig_get_act_tables = None


def _patched_tables(arch):
    full = _orig_get_act_tables(arch)
    keep = ("natural_log_exp_and_others", "sqrt_and_others")
    return {name: (funcs if name in keep else set())
            for name, funcs in full.items()}


def _install_act_table_patch():
    global _orig_get_act_tables
    if _orig_get_act_tables is None:
        _orig_get_act_tables = bacc.get_activation_tables
        bacc.get_activation_tables = _patched_tables


def build_nc(mm_fast=True, m2_fast=True, n_reps=1):
    """Build the SPMD Bass program (identical on every core).

    mm_fast: float32r for the s-path matmuls (4x PE throughput, ~tf32 prec).
    m2_fast: float32r for the agreement matmuls only.
    Default (both False) is full-fp32 compute, absmax rel err ~6e-6.
    n_reps: unroll the whole routing pass this many times (serialized via a
    data dependency through b_ij) for repetition-slope device timing.
    """
    _install_act_table_patch()
    nc = bacc.Bacc("TRN2", target_bir_lowering=False, debug=False,
                   num_devices=NCORES)

    m2_fast = m2_fast or mm_fast
    NP = 256 if mm_fast else UJ      # rhs/psum width, s-path matmuls
    NPM = 256 if m2_fast else UJ     # rhs/psum width, agreement matmuls
    DTX = F32R if mm_fast else F32
    DTM = F32R if m2_fast else F32

    x_ci_d = nc.dram_tensor("x_ci", [CI, B], DTX, kind="ExternalInput")
    x_bt_d = nc.dram_tensor("x_bt", [B, CI], DTM, kind="ExternalInput")
    w_s_d = nc.dram_tensor("w_s", [CI, UJ], F32, kind="ExternalInput")
    w01_d = nc.dram_tensor("w01", [CI, NP], DTX, kind="ExternalInput")
    ei_d = nc.dram_tensor("ei", [128, 16], F32, kind="ExternalInput")
    ebc_d = nc.dram_tensor("ebc", [16, 128], F32, kind="ExternalInput")
    v_out_d = nc.dram_tensor("v_out", [BSH, UJ], F32, kind="ExternalOutput")

    rg = [list(range(NCORES))]

    with tile.TileContext(nc) as tc:
        with (
            tc.tile_pool(name="persist", bufs=1) as pp,
            tc.tile_pool(name="scratch", bufs=3) as sp,
            tc.tile_pool(name="ps_s", bufs=2, space="PSUM") as ps_s,
            tc.tile_pool(name="ps_m2", bufs=2, space="PSUM") as ps_m2,
            tc.tile_pool(name="ps_sm", bufs=2, space="PSUM") as ps_sm,
            tc.tile_pool(name="ps_u", bufs=1, space="PSUM") as ps_u,
            tc.tile_pool(name="dram", bufs=2, space="DRAM") as dp,
        ):
            # ---- persistent SBUF ----
            xci = pp.tile([128, KT * B], DTX, tag="xci")    # [:, k*256:+256]
            xbt = pp.tile([128, BT * CI], DTM, tag="xbt")   # [:, t*1152:+1152]
            ws = pp.tile([128, KT * UJ], F32, tag="ws")     # [:, k*160:+160]
            w01 = pp.tile([128, KT * NP], DTX, tag="w01")   # 0.1*ws (padded)
            weff = pp.tile([128, KT * NP], DTX, tag="weff")
            ei = pp.tile([128, 16], F32, tag="ei")
            ebc = pp.tile([16, 128], F32, tag="ebc")
            b_sb = pp.tile([16, NCH * U], F32, tag="b")     # b_ij [16, 90]
            s_sb = pp.tile([128, BT * UJ], F32, tag="s")    # AR input stage
            sf_sb = pp.tile([128, BT * UJ], F32, tag="sf")  # AR output
            v_sb = pp.tile([128, BT * NPM], DTM, tag="v")
            pj = pp.tile([128, NCH * U], F32, tag="pj")     # j-reduced W*M2

            # DRAM bounce buffers for the collectives are (re)allocated per
            # repetition from a bufs=2 pool: without this, rep r+1's first
            # cc_in write carries a WAR hazard against rep r's final
            # ReduceScatter read, serializing reps end-to-end
            cc_in = cc_out = rs_mid = rs_out = None

            # ---- input loads (per-Ktile so compute starts early) ----
            for k in range(KT):
                nc.sync.dma_start(xci[:, k * B:(k + 1) * B],
                                  x_ci_d[k * 128:(k + 1) * 128, :])
            nc.sync.dma_start(
                xbt[:].rearrange("p (t n) -> p t n", t=BT),
                x_bt_d[:].rearrange("(t p) n -> p t n", p=128))
            nc.sync.dma_start(
                ws[:].rearrange("p (k n) -> p k n", k=KT),
                w_s_d[:].rearrange("(k p) n -> p k n", p=128))
            for k in range(KT):
                nc.sync.dma_start(w01[:, k * NP:(k + 1) * NP],
                                  w01_d[k * 128:(k + 1) * 128, :])
            nc.sync.dma_start(ei[:], ei_d[:])
            nc.sync.dma_start(ebc[:], ebc_d[:])

            if NP > UJ:
                # one-time: zero the f32r pad columns (160..NP) that the
                # widened matmuls read but squash/build_weff never write
                # (f32r memset fails walrus ISA checks -> write via f32 view)
                nc.vector.memset(weff[:].bitcast(F32), 0.0)
            if NPM > UJ:
                nc.vector.memset(v_sb[:].bitcast(F32), 0.0)

            def softmax_to_cj():
                """c_ij = softmax_u(b_ij).  No max-subtraction: |b_ij| < 1."""
                cj = sp.tile([16, NCH * U], F32, tag="cj")
                sm = sp.tile([16, NCH], F32, tag="sm")
                nc.scalar.activation(cj[:], b_sb[:], AF.Exp)
                nc.vector.reduce_sum(
                    sm[:].unsqueeze(2),
                    cj[:].rearrange("p (n u) -> p n u", n=NCH), axis=AX.X)
                nc.vector.reciprocal(sm[:], sm[:])
                nc.vector.tensor_mul(
                    cj[:].rearrange("p (n u) -> p n u", n=NCH),
                    cj[:].rearrange("p (n u) -> p n u", n=NCH),
                    sm[:].unsqueeze(2).to_broadcast((16, NCH, U)))
                return cj

            def build_weff(cj):
                """weff[k] = ws[k] * c_ij broadcast over (i, j).

                partition-broadcast c->(c,i) via a 0/1 selector matmul into
                one PSUM tile, then a single 4D-broadcast DVE multiply."""
                cbc = ps_sm.tile([128, NCH * U], F32, tag="cbc")
                for k in range(KT):
                    nc.tensor.matmul(cbc[:, k * U:(k + 1) * U], ebc[:],
                                     cj[:, k * U:(k + 1) * U],
                                     start=True, stop=True)
                weff4 = (weff[:].rearrange("p (k n) -> p k n", k=KT)
                         [:, :, :UJ].rearrange("p k (u j) -> p k u j", u=U))
                nc.vector.tensor_mul(
                    weff4,
                    ws[:].rearrange("p (k u j) -> p k u j", k=KT, u=U),
                    cbc[:].rearrange("p (k u) -> p k u", k=KT).unsqueeze(3)
                    .to_broadcast((128, KT, U, J)))

            def s_matmul(rhs_tile):
                """s_partial[b,uj] accumulated over KT Ktiles, staged to cc_in."""
                for mt in range(BT):
                    ps = ps_s.tile([128, NP], F32, tag="ps_s")
                    for k in range(KT):
                        nc.tensor.matmul(
                            ps[:],
                            xci[:, k * B + mt * 128: k * B + (mt + 1) * 128],
                            rhs_tile[:, k * NP:(k + 1) * NP],
                            start=(k == 0), stop=(k == KT - 1))
                    nc.vector.tensor_copy(s_sb[:, mt * UJ:(mt + 1) * UJ],
                                          ps[:, :UJ])
                    nc.sync.dma_start(cc_in[mt * 128:(mt + 1) * 128, :],
                                      s_sb[:, mt * UJ:(mt + 1) * UJ])

            def squash(src, dst, parts, T, exact=False):
                """dst = squash_j(src); src [parts, T*UJ], dst 4D [p,T,U,J].

                f = sqrt(m)/(1+m).  The routing iterations use
                f = exp(0.5*ln m - ln(1+m)) so every ACT op stays on the
                Exp/Ln table; the output iteration uses real Sqrt."""
                TU = T * U
                sq = sp.tile([128, BT * UJ], F32, tag="sq")
                mag = sp.tile([128, BT * U], F32, tag="mag")
                lg = sp.tile([128, BT * U], F32, tag="lg")
                l1 = sp.tile([128, BT * U], F32, tag="l1")
                nc.scalar.activation(sq[:parts, :T * UJ], src, AF.Square)
                nc.vector.reduce_sum(
                    mag[:parts, :TU].unsqueeze(2),
                    sq[:parts, :T * UJ].rearrange("p (g j) -> p g j", j=J),
                    axis=AX.X)
                if exact:
                    # g = sqrt(m); f = m/((1+m)*g)
                    nc.scalar.activation(l1[:parts, :TU], mag[:parts, :TU],
                                         AF.Sqrt)
                    nc.vector.scalar_tensor_tensor(
                        l1[:parts, :TU], mag[:parts, :TU], 1.0,
                        l1[:parts, :TU], AluOpType.add, AluOpType.mult)
                    nc.vector.reciprocal(l1[:parts, :TU], l1[:parts, :TU])
                    nc.vector.tensor_mul(lg[:parts, :TU], mag[:parts, :TU],
                                         l1[:parts, :TU])
                else:
                    nc.scalar.activation(lg[:parts, :TU], mag[:parts, :TU],
                                         AF.Ln)
                    nc.scalar.activation(l1[:parts, :TU], mag[:parts, :TU],
                                         AF.Ln, bias=1.0)
                    nc.vector.scalar_tensor_tensor(
                        lg[:parts, :TU], lg[:parts, :TU], 0.5, l1[:parts, :TU],
                        AluOpType.mult, AluOpType.subtract)
                    nc.scalar.activation(lg[:parts, :TU], lg[:parts, :TU],
                                         AF.Exp)
                nc.vector.tensor_mul(
                    dst,
                    src.rearrange("p (t u j) -> p t u j", t=T, u=U),
                    lg[:parts, :TU].rearrange("p (t u) -> p t u", t=T)
                    .unsqueeze(3).to_broadcast((parts, T, U, J)))

            for rep in range(n_reps):
              cc_in = dp.tile([B, UJ], F32, tag="cc_in")
              cc_out = dp.tile([B, UJ], F32, tag="cc_out")
              rs_mid = dp.tile([BSH, UJ], F32, tag="rs_mid")
              rs_out = dp.tile([BSH, UJ], F32, tag="rs_out")
              if rep == 0:
                  nc.vector.memset(b_sb[:], 0.0)
              else:
                  # read-modify-write keeps a data dependency on the previous
                  # repetition so reps serialize for slope timing
                  nc.vector.tensor_scalar_mul(b_sb[:], b_sb[:], 0.0)
              for it in range(3):
                if it == 0:
                    rhs = w01        # softmax(0) = 1/U folded in on the host
                else:
                    cj = softmax_to_cj()
                    build_weff(cj)
                    rhs = weff

                s_matmul(rhs)

                if it < 2:
                    # ReduceScatter + AllGather chained through DRAM beats one
                    # AllReduce: AR pays ~1.875x the per-call floor, RS+AG pay
                    # ~1x each with no extra engine hops between them
                    nc.gpsimd.collective_compute(
                        "ReduceScatter", AluOpType.add, replica_groups=rg,
                        ins=[cc_in[:].opt()], outs=[rs_mid[:].opt()])
                    nc.gpsimd.collective_compute(
                        "AllGather", AluOpType.bypass, replica_groups=rg,
                        ins=[rs_mid[:].opt()], outs=[cc_out[:].opt()])
                    nc.sync.dma_start(
                        sf_sb[:].rearrange("p (t n) -> p t n", t=BT),
                        cc_out[:].rearrange("(t p) n -> p t n", p=128))
                    v4 = (v_sb[:].rearrange("p (t n) -> p t n", t=BT)
                          [:, :, :UJ].rearrange("p t (u j) -> p t u j", u=U))
                    squash(sf_sb[:], v4, 128, BT)

                    # agreement; PSUM tiles packed PK-per-bank so each DVE
                    # mul/reduce instruction covers several m-chunks
                    PK = 512 // NPM
                    for m0 in range(0, KT, PK):
                        mn = min(PK, KT - m0)
                        m2 = ps_m2.tile([128, PK * NPM], F32, tag="m2")
                        for mi in range(mn):
                            m = m0 + mi
                            for t in range(BT):
                                nc.tensor.matmul(
                                    m2[:, mi * NPM:(mi + 1) * NPM],
                                    xbt[:, t * CI + m * 128:
                                        t * CI + (m + 1) * 128],
                                    v_sb[:, t * NPM:(t + 1) * NPM],
                                    start=(t == 0), stop=(t == BT - 1))
                        prod = sp.tile([128, 3 * UJ], F32, tag="prod")
                        nc.vector.tensor_mul(
                            prod[:, :mn * UJ].rearrange(
                                "p (m n) -> p m n", m=mn),
                            ws[:, m0 * UJ:(m0 + mn) * UJ].rearrange(
                                "p (m n) -> p m n", m=mn),
                            m2[:].rearrange("p (m n) -> p m n", m=PK)
                            [:, :mn, :UJ])
                        nc.vector.reduce_sum(
                            pj[:, m0 * U:(m0 + mn) * U].unsqueeze(2),
                            prod[:, :mn * UJ].rearrange(
                                "p (g j) -> p g j", j=J),
                            axis=AX.X)
                    uvj = ps_u.tile([16, NCH * U], F32, tag="uvj")
                    nc.tensor.matmul(uvj[:], ei[:], pj[:],
                                     start=True, stop=True)
                    nc.vector.scalar_tensor_tensor(
                        b_sb[:], uvj[:], 1.0 / B, b_sb[:],
                        AluOpType.mult, AluOpType.add)
                else:
                    nc.gpsimd.collective_compute(
                        "ReduceScatter", AluOpType.add, replica_groups=rg,
                        ins=[cc_in[:].opt()], outs=[rs_out[:].opt()])
                    s32 = sp.tile([BSH, UJ], F32, tag="s32")
                    v32 = sp.tile([BSH, UJ], F32, tag="v32")
                    nc.sync.dma_start(s32[:], rs_out[:])
                    # exact=False keeps every ACT op on the Exp/Ln table: the
                    # Sqrt variant forced a 1.3us act-table swap twice per
                    # pass (sqrt table in, exp/ln table back at next softmax)
                    squash(s32[:],
                           v32[:].rearrange("p (t u j) -> p t u j", t=1, u=U),
                           BSH, 1, exact=False)
                    nc.sync.dma_start(v_out_d[:], v32[:])

    nc.compile()
    return nc


def make_inputs(x, weight, mm_fast=True):
    """Host-side shard prep. x [B,I,C], weight [1,C,U,J,I] (both fp32)."""
    NP = 256 if mm_fast else UJ
    x = np.asarray(x, dtype=np.float32)
    w = np.asarray(weight, dtype=np.float32)[0]       # [C,U,J,I]
    ei = np.zeros((128, 16), np.float32)
    ei[np.arange(128), np.arange(128) // 8] = 1.0
    ebc = np.ascontiguousarray(ei.T)
    in_maps = []
    for k in range(NCORES):
        cs = k * CSH
        xc = x[:, :, cs:cs + CSH]                     # [B, I, CSH]
        x_ci = np.ascontiguousarray(
            xc.transpose(2, 1, 0).reshape(CI, B))     # [(c,i), b]
        x_bt = np.ascontiguousarray(
            xc.transpose(0, 2, 1).reshape(B, CI))     # [b, (c,i)]
        w_s = np.ascontiguousarray(
            w[cs:cs + CSH].transpose(0, 3, 1, 2).reshape(CI, UJ))
        w01 = np.zeros((CI, NP), np.float32)
        w01[:, :UJ] = 0.1 * w_s
        in_maps.append({"x_ci": x_ci, "x_bt": x_bt, "w_s": w_s, "w01": w01,
                        "ei": ei, "ebc": ebc})
    return in_maps


_CACHE = {}


def _get_nc(mm_fast=True, m2_fast=True):
    key = (mm_fast, m2_fast)
    if key not in _CACHE:
        _CACHE[key] = build_nc(mm_fast=mm_fast, m2_fast=m2_fast)
    return _CACHE[key]


def kernel(x, weight, ep=None, **_ignored):
    """Full inputs in, full output out; runs SPMD on 8 NeuronCores."""
    nc = _get_nc()
    in_maps = make_inputs(x, weight)
    res = run_bass_kernel_spmd(nc, in_maps, core_ids=list(range(NCORES)))
    v = np.concatenate([res.results[k]["v_out"] for k in range(NCORES)],
                       axis=0)
    return np.ascontiguousarray(v.reshape(B, U, J, 1))

